# revision 9
# baseline (speedup 1.0000x reference)
"""DCN (DLRM-style deep & cross network) Trainium2 Bass kernel.

Sharding: data-parallel over batch across 8 NeuronCores (2048 samples/core).
Embedding tables (bf16) + MLP weights are replicated to every core's HBM.

Per-core pipeline, quarter-granular (4 quarters x 512 samples):
  1. Embedding gather via InstDMAGatherAnt: one instruction per
     (quarter, category), fetching 256B quad-rows (4 bf16 vocab rows,
     the minimum 256B element) with int16 indices v//4, spread over all
     4 SWDGE queues so the 4 Q7 cpu pairs generate descriptors in
     parallel. The stream is HBM random-read bound (~115 GB/s).
  2. copy + 3x copy_predicated select the right 32-value quarter per
     lookup into the combined feature tile (batch-natural, bf16).
  3. PE 128x128 transposes -> ct[k] = combined^T chunks [128, 512]
  4. MLP: h^T = relu(W^T @ x^T) chains, bf16 matmuls, fp32 accumulate.
  5. CrossNet folds to 4 packed dot products against x0 (alpha0..2,
     Wc_x) plus a scalar chain with host-precomputed alpha_l.b_j consts.
  6. final = sigmoid(x-part + Wc_h . h3 + bc), assembled batch-natural.

Chunk ci's compute depends only on quarter ci's 26 gathers, so compute
chases the gather stream with a ~30us tail instead of half-batch (1024)
granularity; quarter gather time ~= chunk PE time keeps PE warm (HAM).
"""

import numpy as np

import concourse.bass as bass
import concourse.mybir as mybir
import concourse.tile as tile
from concourse import bacc
from concourse.bass import broadcast_tensor_aps
from concourse.bass_utils import run_bass_kernel_spmd
from concourse.masks import make_identity

F32 = mybir.dt.float32
BF16 = mybir.dt.bfloat16
I32 = mybir.dt.int32
I16 = mybir.dt.int16

B = 16384
NCORES = 8
BC = B // NCORES            # 2048 samples per core
NCAT = 26
VOCAB = 100000
EMB = 32
NNUM = 13
D = NCAT * EMB + NNUM       # 845

L1, L2, L3 = 1024, 512, 256
NCROSS = 3
KC = 7                      # feature chunks of 128 (6*128 + 77)
KW = [128] * 6 + [D - 6 * 128]
CPB = 128 // EMB            # 4 categories per 128-feature block
M1, M2, M3 = L1 // 128, L2 // 128, L3 // 128   # 8, 4, 2
NTILE = BC // 128           # 16 batch tiles per core
CHUNK = 512                 # batch chunk (matmul N) == gather quarter
NCHUNK = BC // CHUNK        # 4
TPC = CHUNK // 128          # 4 batch tiles per chunk

QB = CHUNK                  # 512 lookups per (quarter, category)
QCOLS = QB // 16            # 32 idx columns per gather block


def _build(cross_consts) -> bass.Bass:
    # cross_consts = (c10, c20, c21, d0, d1, d2):
    #   c_lj = alpha_l . cross_bias_j,  d_j = Wc_x . cross_bias_j
    c10, c20, c21, d0, d1, d2 = cross_consts

    nc = bacc.Bacc("TRN2", target_bir_lowering=False, num_swdge_queues=4)

    d_emb = nc.dram_tensor("emb", [NCAT * VOCAB, EMB], BF16, kind="ExternalInput")
    d_cat = nc.dram_tensor("cat", [128, NTILE * NCAT], I32, kind="ExternalInput")
    d_gidx = nc.dram_tensor("gidx", [16, NCHUNK * NCAT * QCOLS], I16, kind="ExternalInput")
    d_num = nc.dram_tensor("num", [128, NTILE * NNUM], F32, kind="ExternalInput")
    d_w1 = nc.dram_tensor("w1", [D, L1], BF16, kind="ExternalInput")
    d_w2 = nc.dram_tensor("w2", [L1, L2], BF16, kind="ExternalInput")
    d_w3 = nc.dram_tensor("w3", [L2, L3], BF16, kind="ExternalInput")
    d_b1 = nc.dram_tensor("b1r", [128, M1], F32, kind="ExternalInput")
    d_b2 = nc.dram_tensor("b2r", [128, M2], F32, kind="ExternalInput")
    d_b3 = nc.dram_tensor("b3r", [128, M3], F32, kind="ExternalInput")
    d_bc = nc.dram_tensor("bcr", [128, 1], F32, kind="ExternalInput")
    # avec: per k-chunk 4 columns [alpha0, alpha1, alpha2, wc_x]
    d_avec = nc.dram_tensor("avec", [128, KC * 4], BF16, kind="ExternalInput")
    d_wch = nc.dram_tensor("wch", [128, 2], BF16, kind="ExternalInput")
    d_out = nc.dram_tensor("out", [128, NTILE], F32, kind="ExternalOutput")

    with tile.TileContext(nc) as tc:
        with (
            tc.tile_pool(name="consts", bufs=1) as consts,
            tc.tile_pool(name="quadp", bufs=8) as quadp,
            tc.tile_pool(name="ctp", bufs=2) as ctp,
            tc.tile_pool(name="actp", bufs=3) as actp,
            tc.tile_pool(name="rowp", bufs=1) as rowp,
            tc.tile_pool(name="ps_mm", bufs=3, space="PSUM") as ps_mm,
            tc.tile_pool(name="ps_tp", bufs=3, space="PSUM") as ps_tp,
            tc.tile_pool(name="ps_a", bufs=1, space="PSUM") as ps_a,
            tc.tile_pool(name="ps_h", bufs=1, space="PSUM") as ps_h,
        ):
            # ---------------- constants / inputs ----------------
            gidx = consts.tile([128, NCHUNK * NCAT * QCOLS], I16, name="gidx_sb")
            nc.sync.dma_start(gidx[0:16, :], d_gidx[:])
            for r in range(1, 8):
                nc.sync.dma_start(gidx[r * 16 : (r + 1) * 16, :], gidx[0:16, :])
            cat_sb = consts.tile([128, NTILE * NCAT], I32, name="cat_sb")
            nc.sync.dma_start(cat_sb[:], d_cat[:])
            num_sb = consts.tile([128, NTILE * NNUM], F32, name="num_sb")
            nc.sync.dma_start(num_sb[:], d_num[:])
            avec = consts.tile_from(d_avec[:], name="avec_sb")
            wch = consts.tile_from(d_wch[:], name="wch_sb")

            ident = consts.tile([128, 128], F32, name="ident")
            make_identity(nc, ident)
            ident_bf = consts.tile([128, 128], BF16, name="ident_bf")
            make_identity(nc, ident_bf)
            warm = ps_tp.tile([128, 4], F32, name="warm", tag="pst")
            nc.tensor.transpose(warm[0:4, 0:4], ident[0:4, 0:4], ident[0:4, 0:4])

            # quarter-select masks: m_i = ((cat & 3) == i), i in 1..3
            qq = consts.tile([128, NTILE * NCAT], I32, name="qq")
            nc.vector.tensor_single_scalar(
                qq[:], cat_sb[:], 3, mybir.AluOpType.bitwise_and
            )
            masks = []
            for i in range(1, 4):
                mi = consts.tile([128, NTILE * NCAT], I32, name=f"m{i}")
                nc.vector.tensor_single_scalar(
                    mi[:], qq[:], i, mybir.AluOpType.is_equal
                )
                masks.append(mi[:].rearrange("p (T c) -> p T c", c=NCAT))

            # combined features per (quarter, k-block), batch-natural bf16:
            # block k holds categories 4k..4k+3 (block 6: cats 24,25 + nums)
            cnb = [
                [
                    consts.tile([128, TPC, KW[k]], BF16, name=f"cnb{q}_{k}")
                    for k in range(KC)
                ]
                for q in range(NCHUNK)
            ]

            # natural-layout accumulators for the final combine
            a_nat = consts.tile([128, NTILE * 4], F32, name="a_nat")
            h_nat = consts.tile([128, NTILE], F32, name="h_nat")
            out_nat = consts.tile([128, NTILE], F32, name="out_nat")

            ct_tiles = {}

            def emit_transposes(q, k):
                kw = KW[k]
                ctk = ctp.tile([128, CHUNK], BF16, name=f"ct{k}", tag=f"ct{k}")
                ct_tiles[(q, k)] = ctk
                for t in range(TPC):
                    pst = ps_tp.tile([128, 128], BF16, name="pst", tag="pst")
                    nc.tensor.transpose(
                        pst[0:kw, :],
                        cnb[q][k][:, t, 0:kw],
                        ident_bf[:],
                    )
                    nc.any.tensor_copy(
                        ctk[0:kw, t * 128 : (t + 1) * 128], pst[0:kw, :]
                    )

            def emit_gather_quarter(q):
                # numerical features into block 6 cols 64:77 (ready early)
                for t in range(TPC):
                    T = q * TPC + t
                    nc.any.tensor_copy(
                        cnb[q][6][:, t, 2 * EMB : KW[6]],
                        num_sb[:, T * NNUM : (T + 1) * NNUM],
                    )
                for c in range(NCAT):
                    quad = quadp.tile([128, TPC, 4 * EMB], BF16, name="quad")
                    nc.gpsimd.dma_gather(
                        out_ap=quad[:],
                        in_ap=d_emb[c * VOCAB : (c + 1) * VOCAB, :].rearrange(
                            "(r q) e -> r (q e)", q=4
                        ),
                        idxs_ap=gidx[
                            :,
                            (q * NCAT + c) * QCOLS : (q * NCAT + c + 1) * QCOLS,
                        ],
                        num_idxs=QB,
                        num_idxs_reg=QB,
                        elem_size=4 * EMB,
                        queue_num=(q * NCAT + c) % 4,
                    )
                    co = (c % CPB) * EMB
                    dest = cnb[q][c // CPB][:, :, co : co + EMB]
                    nc.scalar.copy(dest, quad[:, :, 0:EMB])
                    for i in range(1, 4):
                        mslice = masks[i - 1][:, q * TPC : (q + 1) * TPC, c : c + 1]
                        mb, _ = broadcast_tensor_aps(mslice, dest)
                        nc.vector.copy_predicated(
                            dest, mb, quad[:, :, i * EMB : (i + 1) * EMB]
                        )
                    if c % CPB == CPB - 1 and c // CPB < 6:
                        emit_transposes(q, c // CPB)
                    if c == NCAT - 1:
                        emit_transposes(q, 6)

            def emit_chunk(ci):
                ct = [ct_tiles[(ci, k)] for k in range(KC)]
                # cross-net dot products: [alpha0, alpha1, alpha2, wc_x]
                psa = ps_a.tile([4, CHUNK], F32, name="psa", tag="psa")
                for k in range(KC):
                    kw = KW[k]
                    nc.tensor.matmul(
                        psa[:],
                        avec[0:kw, k * 4 : (k + 1) * 4],
                        ct[k][0:kw, :],
                        start=(k == 0),
                        stop=(k == KC - 1),
                    )
                a_sb = actp.tile([4, CHUNK], F32, name="a_sb")
                nc.any.tensor_copy(a_sb[:], psa[:])
                for t in range(TPC):
                    pta = ps_tp.tile([128, 4], F32, name="pta", tag="pst")
                    nc.tensor.transpose(
                        pta[:],
                        a_sb[:, t * 128 : (t + 1) * 128],
                        ident[0:4, 0:4],
                    )
                    T = ci * TPC + t
                    nc.vector.tensor_copy(a_nat[:, T * 4 : (T + 1) * 4], pta[:])

                # MLP
                h1 = []
                for m in range(M1):
                    psm = ps_mm.tile([128, CHUNK], F32, name="psm")
                    for k in range(KC):
                        kw = KW[k]
                        nc.tensor.matmul(
                            psm[:],
                            w1[k][0:kw, m * 128 : (m + 1) * 128],
                            ct[k][0:kw, :],
                            start=(k == 0),
                            stop=(k == KC - 1),
                        )
                    h = actp.tile([128, CHUNK], BF16, name=f"h1_{m}")
                    nc.scalar.activation(
                        h[:], psm[:], mybir.ActivationFunctionType.Relu,
                        bias=b1r[:, m : m + 1],
                    )
                    h1.append(h)
                h2 = []
                for m in range(M2):
                    psm = ps_mm.tile([128, CHUNK], F32, name="psm")
                    for k in range(M1):
                        nc.tensor.matmul(
                            psm[:],
                            w2[k][:, m * 128 : (m + 1) * 128],
                            h1[k][:],
                            start=(k == 0),
                            stop=(k == M1 - 1),
                        )
                    h = actp.tile([128, CHUNK], BF16, name=f"h2_{m}")
                    nc.scalar.activation(
                        h[:], psm[:], mybir.ActivationFunctionType.Relu,
                        bias=b2r[:, m : m + 1],
                    )
                    h2.append(h)
                h3 = []
                for m in range(M3):
                    psm = ps_mm.tile([128, CHUNK], F32, name="psm")
                    for k in range(M2):
                        nc.tensor.matmul(
                            psm[:],
                            w3[k][:, m * 128 : (m + 1) * 128],
                            h2[k][:],
                            start=(k == 0),
                            stop=(k == M2 - 1),
                        )
                    h = actp.tile([128, CHUNK], BF16, name=f"h3_{m}")
                    nc.scalar.activation(
                        h[:], psm[:], mybir.ActivationFunctionType.Identity,
                        bias=b3r[:, m : m + 1],
                    )
                    h3.append(h)

                # h3 . wc_h -> row -> batch-natural
                psh = ps_h.tile([1, CHUNK], F32, name="psh", tag="psrow")
                for j in range(M3):
                    nc.tensor.matmul(
                        psh[:], wch[:, j : j + 1], h3[j][:],
                        start=(j == 0), stop=(j == M3 - 1),
                    )
                h_sb = actp.tile([1, CHUNK], F32, name="h_sb")
                nc.any.tensor_copy(h_sb[:], psh[:])
                for t in range(TPC):
                    pth = ps_tp.tile([128, 1], F32, name="pth", tag="pst")
                    nc.tensor.transpose(
                        pth[:], h_sb[:, t * 128 : (t + 1) * 128], ident[0:1, 0:1]
                    )
                    T = ci * TPC + t
                    nc.vector.tensor_copy(h_nat[:, T : T + 1], pth[:])

            # ------------- final combine (batch-natural, per quarter) ------
            # x3 = p3*x0 + q30*b0 + q31*b1 + b2 with per-sample scalars from
            # the a-dots; Wc_x.x3 folds to p3*awc + q30*d0 + q31*d1 + d2.
            def emit_combine(q):
                NT = TPC
                av = a_nat[:, q * TPC * 4 : (q + 1) * TPC * 4].rearrange(
                    "p (t l) -> p t l", l=4
                )
                a0, a1, a2, awc = (av[:, :, l] for l in range(4))
                hn = h_nat[:, q * TPC : (q + 1) * TPC]

                def rtile(name):
                    return rowp.tile([128, NT], F32, name=name, tag=f"{name}_{q}")

                p1 = rtile("p1")            # 1 + s0
                nc.vector.tensor_scalar_add(p1[:], a0, 1.0)
                s1 = rtile("s1")            # s1 = p1*a1 (+ c10)
                nc.vector.tensor_mul(s1[:], a1, p1[:])
                if c10 != 0.0:
                    nc.vector.tensor_scalar_add(s1[:], s1[:], float(c10))
                u1 = rtile("u1")            # 1 + s1  (= q20)
                nc.vector.tensor_scalar_add(u1[:], s1[:], 1.0)
                p2 = rtile("p2")
                nc.vector.tensor_mul(p2[:], p1[:], u1[:])
                s2 = rtile("s2")            # s2 = p2*a2 + u1*c20 + c21
                nc.vector.tensor_mul(s2[:], a2, p2[:])
                if c20 != 0.0:
                    v20 = rtile("v20")
                    nc.vector.tensor_scalar_mul(v20[:], u1[:], float(c20))
                    nc.vector.tensor_add(s2[:], s2[:], v20[:])
                if c21 != 0.0:
                    nc.vector.tensor_scalar_add(s2[:], s2[:], float(c21))
                u2 = rtile("u2")            # 1 + s2
                nc.vector.tensor_scalar_add(u2[:], s2[:], 1.0)
                p3 = rtile("p3")
                nc.vector.tensor_mul(p3[:], p2[:], u2[:])
                fin = rtile("fin")          # awc*p3 (+ bias-derived terms)
                nc.vector.tensor_mul(fin[:], awc, p3[:])
                if d0 != 0.0:
                    q30 = rtile("q30")
                    nc.vector.tensor_mul(q30[:], u1[:], u2[:])
                    nc.vector.tensor_scalar_mul(q30[:], q30[:], float(d0))
                    nc.vector.tensor_add(fin[:], fin[:], q30[:])
                if d1 != 0.0:
                    w1t = rtile("w1t")
                    nc.vector.tensor_scalar_mul(w1t[:], u2[:], float(d1))
                    nc.vector.tensor_add(fin[:], fin[:], w1t[:])
                if d2 != 0.0:
                    nc.vector.tensor_scalar_add(fin[:], fin[:], float(d2))
                nc.vector.tensor_add(fin[:], fin[:], hn)
                ons = out_nat[:, q * TPC : (q + 1) * TPC]
                nc.scalar.activation(
                    ons, fin[:], mybir.ActivationFunctionType.Sigmoid,
                    bias=bcr[:, 0:1],
                )
                nc.sync.dma_start(d_out[:, q * TPC : (q + 1) * TPC], ons)

            emit_gather_quarter(0)
            w1 = [
                consts.tile_from(d_w1[k * 128 : k * 128 + KW[k], :], name=f"w1_{k}")
                for k in range(KC)
            ]
            w2 = [
                consts.tile_from(d_w2[k * 128 : (k + 1) * 128, :], name=f"w2_{k}")
                for k in range(M1)
            ]
            w3 = [
                consts.tile_from(d_w3[k * 128 : (k + 1) * 128, :], name=f"w3_{k}")
                for k in range(M2)
            ]
            b1r = consts.tile_from(d_b1[:], name="b1r_sb")
            b2r = consts.tile_from(d_b2[:], name="b2r_sb")
            b3r = consts.tile_from(d_b3[:], name="b3r_sb")
            bcr = consts.tile_from(d_bc[:], name="bcr_sb")
            emit_gather_quarter(1)
            emit_chunk(0)
            emit_combine(0)
            for q in range(2, NCHUNK):
                emit_gather_quarter(q)
                emit_chunk(q - 1)
                emit_combine(q - 1)
            emit_chunk(NCHUNK - 1)
            emit_combine(NCHUNK - 1)

    nc.compile()
    return nc


_CACHE: dict = {}


def _get_nc(cross_consts) -> bass.Bass:
    key = cross_consts
    if key not in _CACHE:
        _CACHE[key] = _build(cross_consts)
    return _CACHE[key]


def kernel(
    categorical_input,
    numerical_input,
    emb_tables,
    alphas,
    cross_bias,
    W1, b1, W2, b2, W3, b3, Wc, bc,
) -> np.ndarray:
    cat = np.ascontiguousarray(np.asarray(categorical_input, dtype=np.int64))
    num = np.ascontiguousarray(np.asarray(numerical_input, dtype=np.float32))
    emb = np.ascontiguousarray(
        np.asarray(emb_tables, dtype=np.float32).reshape(NCAT * VOCAB, EMB)
    )
    alphas = np.asarray(alphas, dtype=np.float32)
    cross_bias = np.asarray(cross_bias, dtype=np.float32)
    W1 = np.ascontiguousarray(np.asarray(W1, dtype=np.float32))
    W2 = np.ascontiguousarray(np.asarray(W2, dtype=np.float32))
    W3 = np.ascontiguousarray(np.asarray(W3, dtype=np.float32))
    Wc = np.asarray(Wc, dtype=np.float32)
    b1 = np.asarray(b1, dtype=np.float32)
    b2 = np.asarray(b2, dtype=np.float32)
    b3 = np.asarray(b3, dtype=np.float32)
    bc = np.asarray(bc, dtype=np.float32)

    # host scalar constants folding cross_bias into the per-sample chain
    cross_consts = (
        float(np.dot(alphas[1], cross_bias[0])),
        float(np.dot(alphas[2], cross_bias[0])),
        float(np.dot(alphas[2], cross_bias[1])),
        float(np.dot(Wc[:D, 0], cross_bias[0])),
        float(np.dot(Wc[:D, 0], cross_bias[1])),
        float(np.dot(Wc[:D, 0], cross_bias[2])),
    )
    nc = _get_nc(cross_consts)

    def to_dev(v):  # [D(,k)] -> [KC*128(,k)] zero-padded
        shape = (KC * 128,) + v.shape[1:]
        p = np.zeros(shape, np.float32)
        p[:D] = v
        return p

    def pad_col(v):  # [845] -> [128, KC] column-chunked, zero-padded
        return to_dev(v).reshape(KC, 128).T.copy()

    avec = np.zeros((128, KC * 4), np.float32)
    for l in range(NCROSS):
        avec[:, l::4] = pad_col(alphas[l])
    avec[:, 3::4] = pad_col(Wc[:D, 0])
    wch = Wc[D : D + L3, 0].reshape(2, 128).T.copy()
    b1r = b1.reshape(M1, 128).T.copy()
    b2r = b2.reshape(M2, 128).T.copy()
    b3r = b3.reshape(M3, 128).T.copy()
    bcr = np.broadcast_to(bc.reshape(1, 1), (128, 1)).copy()

    import ml_dtypes

    bf = ml_dtypes.bfloat16
    common = {
        "emb": emb.astype(bf),
        "w1": W1.astype(bf),
        "w2": W2.astype(bf),
        "w3": W3.astype(bf),
        "b1r": b1r,
        "b2r": b2r,
        "b3r": b3r,
        "bcr": bcr,
        "avec": avec.astype(bf),
        "wch": wch.astype(bf),
    }
    in_maps = []
    for core in range(NCORES):
        cs = cat[core * BC : (core + 1) * BC].astype(np.int32)  # [2048, 26]
        ns = num[core * BC : (core + 1) * BC]
        catr = np.ascontiguousarray(
            cs.reshape(NTILE, 128, NCAT).transpose(1, 0, 2).reshape(128, NTILE * NCAT)
        )
        numr = np.ascontiguousarray(
            ns.reshape(NTILE, 128, NNUM).transpose(1, 0, 2).reshape(128, NTILE * NNUM)
        )
        # gather indices: per (quarter, category) block of QCOLS cols,
        # int16 v//4, lookup i at [i % 16, i // 16]
        gi = np.zeros((16, NCHUNK * NCAT * QCOLS), np.int16)
        for q in range(NCHUNK):
            vs = cs[q * QB : (q + 1) * QB]  # [512, 26]
            q4 = (vs // 4).astype(np.int16)
            wrapped = q4.reshape(QCOLS, 16, NCAT).transpose(1, 0, 2)  # [16,32,26]
            for c in range(NCAT):
                blk = (q * NCAT + c) * QCOLS
                gi[:, blk : blk + QCOLS] = wrapped[:, :, c]
        in_maps.append({**common, "cat": catr, "num": numr, "gidx": gi})

    res = run_bass_kernel_spmd(nc, in_maps, core_ids=list(range(NCORES)))
    outs = []
    for core in range(NCORES):
        o = res.results[core]["out"]  # [128, NTILE], sample T*128+p at [p, T]
        outs.append(o.T.reshape(BC, 1))
    return np.concatenate(outs, axis=0).astype(np.float32)
